# revision 3
# baseline (speedup 1.0000x reference)
"""SAGEConv(aggr='max') Trainium2 kernel, sharded over 8 NeuronCores.

Problem:  out_i = W_l @ max_{j in N(i)} x_j + b_l + W_r @ x_i
          X [50000,128] f32, edge_index [2,800000] int64, out [50000,1] f32.

Strategy (dst-sharded, 8 cores), v2 — bf16 slot-major streaming:
  - Each core owns 6250 destination nodes; edges are partitioned by dst.
  - Host sorts each core's nodes by in-degree (descending) into 49 tiles of
    128 nodes; tile t has K_t = max in-tile degree slots per node (shared
    across cores via elementwise max so one SPMD program serves all).
    Degree-descending order makes K_t non-increasing, so for every slot
    index k the set {t : K_t > k} is a prefix of the tiles.
  - Host lays the neighbor table out SLOT-MAJOR in bf16: region k holds
    [128, n_k*128] where n_k = #tiles with K_t > k (pure index-driven row
    permutation + dtype cast of X; pad slots duplicate the node's first
    edge — max is idempotent — and degree-0 nodes get zero rows, matching
    PyG's empty-segment fill).
  - Device: slot 0 DMAs straight into the accumulator acc=[128, 49*128];
    slots k>=1 stream through a ring of SBUF buffers (grouped into ~1.5MB
    DMAs) and fold in with ONE vector max per slot over a [128, n_k*128]
    prefix (bf16 hits the DVE 2x perf mode).  After slot k the tiles with
    K_t == k+1 are final, so their two fused dot-products (agg.W_l and
    [own|1].(W_r|b_l), accum_out into f32 columns) issue immediately and
    hide under the DMA stream.  A final f32 add merges the two partial
    result rows; one small store at the end.
  - bf16 halves HBM traffic vs f32 (28MB/core vs 56MB/core) and doubles
    DVE throughput; the kernel is DMA-bound at the ~358GB/s/core HBM
    roofline (~78us predicted vs 157us for the f32 baseline).
  - Host unpermutes per-core outputs back to global node order.
"""

import numpy as np
import ml_dtypes

N_NODES = 50000
N_EDGES = 800000
D_IN = 128
N_CORES = 8
NPC = N_NODES // N_CORES  # 6250 nodes per core
P = 128
NT = (NPC + P - 1) // P  # 49 tiles of 128 nodes
NODES_PAD = NT * P  # 6272
DF = 132  # own-feature block: 128 dims + 1 bias-one column + 3 pad

F32 = np.float32
BF16 = ml_dtypes.bfloat16

NRING = 8  # ring depth for streaming slot-group buffers
GROUP_TARGET = 6272  # elems/partition per DMA group (= 1.57MB at bf16)


# ---------------------------------------------------------------- host side
def _plan(K_prog):
    """Slot widths n_k and DMA groups [(k_lo, k_hi), ...] over slots >=1."""
    Kmax = int(K_prog[0])
    n_k = [int((K_prog > k).sum()) for k in range(Kmax)]
    groups = []
    k = 1
    while k < Kmax:
        lo = k
        w = 0
        while k < Kmax and (w == 0 or w + n_k[k] * 128 <= GROUP_TARGET):
            w += n_k[k] * 128
            k += 1
        groups.append((lo, k))
    return Kmax, n_k, groups


def _preprocess(X, W_l, b_l, W_r, edge_index):
    X = np.asarray(X, dtype=F32)
    W_l = np.asarray(W_l, dtype=F32).reshape(-1)
    W_r = np.asarray(W_r, dtype=F32).reshape(-1)
    b_l = float(np.asarray(b_l).reshape(-1)[0])

    src = np.asarray(edge_index[0], dtype=np.int64)
    dst = np.asarray(edge_index[1], dtype=np.int64)
    core = dst // NPC

    # X in bf16 with a trailing all-zero row: slot index N_NODES = "empty".
    xz = np.zeros((N_NODES + 1, D_IN), dtype=BF16)
    xz[:N_NODES] = X.astype(BF16)

    per_core = []
    K_tiles = np.zeros((N_CORES, NT), dtype=np.int64)
    for c in range(N_CORES):
        sel = core == c
        s = src[sel]
        d = dst[sel] - c * NPC
        deg = np.bincount(d, minlength=NPC)
        order = np.argsort(-deg, kind="stable")  # local ids, degree desc
        deg_sorted = np.zeros(NODES_PAD, dtype=np.int64)
        deg_sorted[:NPC] = deg[order]
        K_tiles[c] = deg_sorted.reshape(NT, P).max(axis=1)

        eorder = np.argsort(d, kind="stable")
        d_s = d[eorder]
        s_s = s[eorder]
        start = np.zeros(NPC + 1, dtype=np.int64)
        np.cumsum(deg, out=start[1:])
        rank = np.arange(len(d_s), dtype=np.int64) - start[d_s]
        ipos = np.empty(NPC, dtype=np.int64)  # local id -> sorted position
        ipos[order] = np.arange(NPC)
        per_core.append((order, deg_sorted, ipos[d_s], rank, s_s))

    K_prog = np.maximum(K_tiles.max(axis=0), 1).astype(np.int64)
    assert (np.diff(K_prog) <= 0).all()
    Kmax, n_k, groups = _plan(K_prog)
    W_slots = sum(n_k) * D_IN  # total slot elems per partition

    in_maps = []
    orders = []
    for c in range(N_CORES):
        order, deg_sorted, pos_e, rank_e, s_s = per_core[c]
        table = np.full((NODES_PAD, Kmax), N_NODES, dtype=np.int64)
        table[pos_e, rank_e] = s_s
        dup = table[:, 0]  # first edge src, or zero-row for degree-0 nodes
        cols = np.arange(Kmax, dtype=np.int64)[None, :]
        table = np.where(cols < deg_sorted[:, None], table, dup[:, None])

        # slot-major neighbor table [P, sum_k n_k*128] bf16
        xg = np.empty((P, W_slots), dtype=BF16)
        off = 0
        for k in range(Kmax):
            n = n_k[k]
            tbl = table[: n * P, k].reshape(n, P).T  # [P, n]
            xg[:, off : off + n * D_IN] = xz[tbl].reshape(P, n * D_IN)
            off += n * D_IN

        # own-feature blocks [P, NT*132]: [x_own 128 | 1 | pad3] per tile
        xown = np.zeros((NODES_PAD, DF), dtype=BF16)
        xown[:NPC, :D_IN] = X[c * NPC + order].astype(BF16)
        xown[:, D_IN] = 1.0
        xo = np.ascontiguousarray(
            xown.reshape(NT, P, DF).transpose(1, 0, 2).reshape(P, NT * DF)
        )

        wagg = np.broadcast_to(W_l.astype(BF16), (P, D_IN)).copy()
        wo = np.zeros((P, DF), dtype=BF16)
        wo[:, :D_IN] = W_r.astype(BF16)[None, :]
        wo[:, D_IN] = b_l

        in_maps.append({"xg": xg, "xo": xo, "wagg": wagg, "wo": wo})
        orders.append(order)

    return in_maps, orders, K_prog


def _assemble(results, orders):
    out = np.empty((N_NODES, 1), dtype=F32)
    for c in range(N_CORES):
        oc = np.asarray(results[c]["out"])  # [P, NT]
        vals = oc.T.reshape(-1)[:NPC]  # sorted-position order
        out[c * NPC + orders[c], 0] = vals
    return out


# -------------------------------------------------------------- device side
def _build_program(K_prog):
    import concourse.bass as bass
    import concourse.mybir as mybir
    from contextlib import ExitStack

    f32 = mybir.dt.float32
    bf16 = mybir.dt.bfloat16
    Kmax, n_k, groups = _plan(K_prog)
    W_slots = sum(n_k) * D_IN
    W_acc = NT * D_IN  # 6272
    # slot k's column offset inside xg
    offs = np.zeros(Kmax + 1, dtype=np.int64)
    np.cumsum(np.asarray(n_k) * D_IN, out=offs[1:])

    nc = bass.Bass()
    xg = nc.declare_dram_parameter("xg", [P, W_slots], bf16, isOutput=False)
    xo = nc.declare_dram_parameter("xo", [P, NT * DF], bf16, isOutput=False)
    wagg_d = nc.declare_dram_parameter("wagg", [P, D_IN], bf16, isOutput=False)
    wo_d = nc.declare_dram_parameter("wo", [P, DF], bf16, isOutput=False)
    out = nc.declare_dram_parameter("out", [P, NT], f32, isOutput=True)

    with ExitStack() as ctx:
        block = ctx.enter_context(nc.Block())
        s_w = ctx.enter_context(nc.semaphore("s_w"))
        s_v = ctx.enter_context(nc.semaphore("s_v"))
        s_out = ctx.enter_context(nc.semaphore("s_out"))
        s_fin = ctx.enter_context(nc.semaphore("s_fin"))
        sg = [ctx.enter_context(nc.semaphore(f"sg{b}")) for b in range(NRING)]

        w_t = ctx.enter_context(nc.sbuf_tensor("w_t", [P, D_IN], bf16))
        wo_t = ctx.enter_context(nc.sbuf_tensor("wo_t", [P, DF], bf16))
        acc = ctx.enter_context(nc.sbuf_tensor("acc", [P, W_acc], bf16))
        xo_t = ctx.enter_context(nc.sbuf_tensor("xo_t", [P, NT * DF], bf16))
        r1 = ctx.enter_context(nc.sbuf_tensor("r1", [P, NT], f32))
        r2 = ctx.enter_context(nc.sbuf_tensor("r2", [P, NT], f32))
        out_acc = ctx.enter_context(nc.sbuf_tensor("out_acc", [P, NT], f32))
        junk = ctx.enter_context(nc.sbuf_tensor("junk", [P, DF], bf16))
        gq = [
            ctx.enter_context(nc.sbuf_tensor(f"gq{b}", [P, GROUP_TARGET], bf16))
            for b in range(NRING)
        ]

        NG = len(groups)

        @block.sync
        def _(sync):
            sync.dma_start(out=w_t[:], in_=wagg_d[:]).then_inc(s_w, 16)
            sync.dma_start(out=wo_t[:], in_=wo_d[:]).then_inc(s_w, 16)
            sync.dma_start(out=xo_t[:], in_=xo[:]).then_inc(s_w, 16)
            # slot 0 straight into the accumulator
            sync.dma_start(
                out=acc[:], in_=xg[:, : int(offs[1])]
            ).then_inc(s_w, 16)
            for g, (lo, hi) in enumerate(groups):
                b = g % NRING
                if g >= NRING:
                    sync.wait_ge(s_v, g - NRING + 1)
                width = int(offs[hi] - offs[lo])
                sync.dma_start(
                    out=gq[b][:, :width],
                    in_=xg[:, int(offs[lo]) : int(offs[hi])],
                ).then_inc(sg[b], 16)
            sync.wait_ge(s_fin, 1)
            sync.dma_start(out=out[:], in_=out_acc[:]).then_inc(s_out, 16)
            sync.wait_ge(s_out, 16)

        @block.vector
        def _(v):
            def dots(t):
                # r1[:,t] = agg_t . W_l ; r2[:,t] = own_t . (W_r|b_l)
                v.scalar_tensor_tensor(
                    out=junk[:, :D_IN],
                    in0=acc[:, t * D_IN : (t + 1) * D_IN],
                    scalar=1.0,
                    in1=w_t[:],
                    op0=mybir.AluOpType.mult,
                    op1=mybir.AluOpType.mult,
                    accum_out=r1[:, t : t + 1],
                )
                v.scalar_tensor_tensor(
                    out=junk[:],
                    in0=xo_t[:, t * DF : (t + 1) * DF],
                    scalar=1.0,
                    in1=wo_t[:],
                    op0=mybir.AluOpType.mult,
                    op1=mybir.AluOpType.mult,
                    accum_out=r2[:, t : t + 1],
                )

            v.wait_ge(s_w, 64)
            # tiles whose K_t == 1 are final right after slot 0
            for t in range(n_k[1] if Kmax > 1 else 0, NT):
                dots(t)
            use = [0] * NRING
            for g, (lo, hi) in enumerate(groups):
                b = g % NRING
                use[b] += 1
                v.wait_ge(sg[b], 16 * use[b])
                for k in range(lo, hi):
                    w = n_k[k] * D_IN
                    goff = int(offs[k] - offs[lo])
                    ins = v.tensor_tensor(
                        out=acc[:, :w],
                        in0=acc[:, :w],
                        in1=gq[b][:, goff : goff + w],
                        op=mybir.AluOpType.max,
                    )
                    if k == hi - 1:
                        ins.then_inc(s_v, 1)
                # tiles with K_t in (lo, hi] are now final: t in [n_hi, n_lo)
                hi_n = n_k[hi] if hi < Kmax else 0
                for t in range(hi_n, n_k[lo]):
                    dots(t)
            # spacers so the last accum_out fully drains before the add
            for _ in range(4):
                v.tensor_copy(out=junk[:, :D_IN], in_=w_t[:])
            v.tensor_tensor(
                out=out_acc[:],
                in0=r1[:],
                in1=r2[:],
                op=mybir.AluOpType.add,
            ).then_inc(s_fin, 1)

    return nc


# ---------------------------------------------------------------- entry
def _run(inputs, trace=False, trace_cores=None):
    from concourse.bass_utils import run_bass_kernel_spmd

    in_maps, orders, K_prog = _preprocess(**inputs)
    nc = _build_program(K_prog)
    res = run_bass_kernel_spmd(
        nc,
        in_maps,
        core_ids=list(range(N_CORES)),
        trace=trace,
        trace_cores=trace_cores,
    )
    return _assemble(res.results, orders), res


def kernel(**inputs):
    out, _ = _run(inputs)
    return out


# revision 6
# speedup vs baseline: 1.1918x; 1.1918x over previous
"""SAGEConv(aggr='max') Trainium2 kernel, sharded over 8 NeuronCores.

Problem:  out_i = W_l @ max_{j in N(i)} x_j + b_l + W_r @ x_i
          X [50000,128] f32, edge_index [2,800000] int64, out [50000,1] f32.

Strategy (dst-sharded, 8 cores), v3 — bf16 transposed slot-major streaming:
  - Each core owns 6250 destination nodes; edges are partitioned by dst.
  - Host sorts each core's nodes by in-degree (descending); tile t of 128
    nodes has K_t = max in-tile degree (shared across cores via elementwise
    max so one SPMD program serves all).  Degree-descending order makes K_t
    non-increasing, so {t : K_t > k} is a prefix for every slot k.
  - TRANSPOSED layout: feature dims live on the 128 SBUF partitions and
    nodes on the free axis.  Host emits slot-region k as [128 dims,
    n_k*128 nodes] bf16 (n_k = #tiles with K_t > k; pure index-driven
    permutation + cast of X; pad slots duplicate the node's first edge —
    max is idempotent — and degree-0 nodes get zero rows = PyG fill).
  - Device dataflow (engines in parallel):
      DMA   : slot 0 straight into acc=[128, 6272]; slots k>=1 stream
              through a ring of SBUF buffers in ~1.5MB grouped transfers.
      DVE   : ONE bf16 max per slot over the [128, n_k*128] prefix (2x
              perf mode) — this is the only vector work.
      PE    : per 512-node chunk, W_l.agg + W_r.own via two accumulating
              [128]x[128,512] matmuls into PSUM (chunks issue as soon as
              their tiles' slots are folded; low-K chunks finish first).
      ACT   : drains each PSUM chunk to the [1, 6272] f32 output row,
              adding b_l via the activation bias.
  - bf16 halves HBM traffic vs f32 (28MB/core) and doubles DVE max
    throughput; kernel is DMA-bound near the per-core HBM roofline.
  - Host unpermutes the per-core output rows back to global node order.
"""

import numpy as np
import ml_dtypes

N_NODES = 50000
N_EDGES = 800000
D_IN = 128
N_CORES = 8
NPC = N_NODES // N_CORES  # 6250 nodes per core
P = 128
NT = (NPC + P - 1) // P  # 49 tiles of 128 nodes
NODES_PAD = NT * P  # 6272

F32 = np.float32
BF16 = ml_dtypes.bfloat16

NRING = 8  # ring depth for streaming slot-group buffers
GROUP_TARGET = 6272  # elems/partition per DMA group (= 1.57MB at bf16)
CHUNK = 512  # nodes per PE/PSUM chunk
NCH = (NODES_PAD + CHUNK - 1) // CHUNK  # 13
NPSUM = 4  # psum ring depth


# ---------------------------------------------------------------- host side
def _plan(K_prog):
    """Slot widths n_k and DMA groups [(k_lo, k_hi), ...] over slots >=1."""
    Kmax = int(K_prog[0])
    n_k = [int((K_prog > k).sum()) for k in range(Kmax)]
    groups = []
    k = 1
    while k < Kmax:
        lo = k
        w = 0
        while k < Kmax and (w == 0 or w + n_k[k] * P <= GROUP_TARGET):
            w += n_k[k] * P
            k += 1
        groups.append((lo, k))
    return Kmax, n_k, groups


def _preprocess(X, W_l, b_l, W_r, edge_index):
    X = np.asarray(X, dtype=F32)
    W_l = np.asarray(W_l, dtype=F32).reshape(-1)
    W_r = np.asarray(W_r, dtype=F32).reshape(-1)
    b_l = float(np.asarray(b_l).reshape(-1)[0])

    src = np.asarray(edge_index[0], dtype=np.int64)
    dst = np.asarray(edge_index[1], dtype=np.int64)
    core = dst // NPC

    # X^T in bf16 with a trailing all-zero column: index N_NODES = "empty".
    xzT = np.zeros((D_IN, N_NODES + 1), dtype=BF16)
    xzT[:, :N_NODES] = X.T.astype(BF16)

    per_core = []
    K_tiles = np.zeros((N_CORES, NT), dtype=np.int64)
    for c in range(N_CORES):
        sel = core == c
        s = src[sel]
        d = dst[sel] - c * NPC
        deg = np.bincount(d, minlength=NPC)
        order = np.argsort(-deg, kind="stable")  # local ids, degree desc
        deg_sorted = np.zeros(NODES_PAD, dtype=np.int64)
        deg_sorted[:NPC] = deg[order]
        K_tiles[c] = deg_sorted.reshape(NT, P).max(axis=1)

        eorder = np.argsort(d, kind="stable")
        d_s = d[eorder]
        s_s = s[eorder]
        start = np.zeros(NPC + 1, dtype=np.int64)
        np.cumsum(deg, out=start[1:])
        rank = np.arange(len(d_s), dtype=np.int64) - start[d_s]
        ipos = np.empty(NPC, dtype=np.int64)  # local id -> sorted position
        ipos[order] = np.arange(NPC)
        per_core.append((order, deg_sorted, ipos[d_s], rank, s_s))

    K_prog = np.maximum(K_tiles.max(axis=0), 1).astype(np.int64)
    assert (np.diff(K_prog) <= 0).all()
    Kmax, n_k, groups = _plan(K_prog)
    W_slots = sum(n_k) * P  # total slot elems per partition row

    in_maps = []
    orders = []
    for c in range(N_CORES):
        order, deg_sorted, pos_e, rank_e, s_s = per_core[c]
        table = np.full((NODES_PAD, Kmax), N_NODES, dtype=np.int64)
        table[pos_e, rank_e] = s_s
        dup = table[:, 0]  # first edge src, or zero-col for degree-0 nodes
        cols = np.arange(Kmax, dtype=np.int64)[None, :]
        table = np.where(cols < deg_sorted[:, None], table, dup[:, None])

        # slot-major transposed neighbor table [128 dims, sum_k n_k*128]
        xg = np.empty((P, W_slots), dtype=BF16)
        off = 0
        for k in range(Kmax):
            n = n_k[k]
            xg[:, off : off + n * P] = xzT[:, table[: n * P, k]]
            off += n * P

        # own features transposed [128 dims, NODES_PAD]
        xo = np.zeros((P, NODES_PAD), dtype=BF16)
        xo[:, :NPC] = xzT[:, c * NPC + order]

        w2 = np.zeros((P, 2), dtype=BF16)
        w2[:, 0] = W_l.astype(BF16)
        w2[:, 1] = W_r.astype(BF16)

        in_maps.append({"xg": xg, "xo": xo, "w2": w2})
        orders.append(order)

    return in_maps, orders, K_prog, b_l


def _assemble(results, orders):
    out = np.empty((N_NODES, 1), dtype=F32)
    for c in range(N_CORES):
        oc = np.asarray(results[c]["out"]).reshape(-1)  # [NODES_PAD]
        out[c * NPC + orders[c], 0] = oc[:NPC]
    return out


# -------------------------------------------------------------- device side
def _build_program(K_prog, b_l):
    import concourse.bass as bass
    import concourse.mybir as mybir
    from contextlib import ExitStack

    f32 = mybir.dt.float32
    bf16 = mybir.dt.bfloat16
    Kmax, n_k, groups = _plan(K_prog)
    W_slots = sum(n_k) * P
    W_acc = NT * P  # 6272
    offs = np.zeros(Kmax + 1, dtype=np.int64)
    np.cumsum(np.asarray(n_k) * P, out=offs[1:])
    NG = len(groups)

    # chunk ci needs slots < K_prog[first tile of chunk] folded in
    chunk_need = []
    for ci in range(NCH):
        t0 = (ci * CHUNK) // P
        chunk_need.append(max(int(K_prog[t0]) - 1, 0))
    chunk_order = list(reversed(range(NCH)))  # high chunks ready first

    nc = bass.Bass()
    xg = nc.declare_dram_parameter("xg", [P, W_slots], bf16, isOutput=False)
    xo = nc.declare_dram_parameter("xo", [P, NODES_PAD], bf16, isOutput=False)
    w2_d = nc.declare_dram_parameter("w2", [P, 2], bf16, isOutput=False)
    out = nc.declare_dram_parameter("out", [1, NODES_PAD], f32, isOutput=True)

    with ExitStack() as ctx:
        block = ctx.enter_context(nc.Block())
        s_a0 = ctx.enter_context(nc.semaphore("s_a0"))  # slot0 -> acc landed
        s_w = ctx.enter_context(nc.semaphore("s_w"))  # w2 + xo landed
        s_v = ctx.enter_context(nc.semaphore("s_v"))  # chain ops completed
        s_p = ctx.enter_context(nc.semaphore("s_p"))  # PE chunk pairs done
        s_ad = ctx.enter_context(nc.semaphore("s_ad"))  # ACT chunks drained
        s_out = ctx.enter_context(nc.semaphore("s_out"))
        sg = [ctx.enter_context(nc.semaphore(f"sg{b}")) for b in range(NRING)]

        w_t = ctx.enter_context(nc.sbuf_tensor("w_t", [P, 2], bf16))
        acc = ctx.enter_context(nc.sbuf_tensor("acc", [P, W_acc], bf16))
        xo_t = ctx.enter_context(nc.sbuf_tensor("xo_t", [P, NODES_PAD], bf16))
        orow = ctx.enter_context(nc.sbuf_tensor("orow", [1, NODES_PAD], f32))
        gq = [
            ctx.enter_context(nc.sbuf_tensor(f"gq{b}", [P, GROUP_TARGET], bf16))
            for b in range(NRING)
        ]
        ps = [
            ctx.enter_context(nc.psum_tensor(f"ps{i}", [1, CHUNK], f32))
            for i in range(NPSUM)
        ]

        @block.sync
        def _(sync):
            # slot 0 leads so the DVE chain can start ASAP
            sync.dma_start(out=acc[:], in_=xg[:, : int(offs[1])]).then_inc(
                s_a0, 16
            )
            for g, (lo, hi) in enumerate(groups):
                b = g % NRING
                if g >= NRING:
                    # ring slot free once its last chain op retired
                    sync.wait_ge(s_v, groups[g - NRING][1] - 1)
                width = int(offs[hi] - offs[lo])
                sync.dma_start(
                    out=gq[b][:, :width],
                    in_=xg[:, int(offs[lo]) : int(offs[hi])],
                ).then_inc(sg[b], 16)
                if g == 0:
                    sync.dma_start(out=w_t[:], in_=w2_d[:]).then_inc(s_w, 16)
                    sync.dma_start(out=xo_t[:], in_=xo[:]).then_inc(s_w, 16)
            sync.wait_ge(s_ad, NCH)
            sync.dma_start(out=out[:], in_=orow[:]).then_inc(s_out, 16)
            sync.wait_ge(s_out, 16)

        @block.vector
        def _(v):
            v.wait_ge(s_a0, 16)
            use = [0] * NRING
            for g, (lo, hi) in enumerate(groups):
                b = g % NRING
                use[b] += 1
                v.wait_ge(sg[b], 16 * use[b])
                for k in range(lo, hi):
                    w = n_k[k] * P
                    goff = int(offs[k] - offs[lo])
                    v.tensor_tensor(
                        out=acc[:, :w],
                        in0=acc[:, :w],
                        in1=gq[b][:, goff : goff + w],
                        op=mybir.AluOpType.max,
                    ).then_inc(s_v, 1)

        @block.tensor
        def _(te):
            te.wait_ge(s_w, 32)
            for i, ci in enumerate(chunk_order):
                c0 = ci * CHUNK
                c1 = min(c0 + CHUNK, NODES_PAD)
                wdt = c1 - c0
                if chunk_need[ci] > 0:
                    te.wait_ge(s_v, chunk_need[ci])
                if i >= NPSUM:
                    te.wait_ge(s_ad, i - NPSUM + 1)
                pb = ps[i % NPSUM]
                te.matmul(
                    pb[:, :wdt],
                    w_t[:, 0:1],
                    acc[:, c0:c1],
                    start=True,
                    stop=False,
                )
                te.matmul(
                    pb[:, :wdt],
                    w_t[:, 1:2],
                    xo_t[:, c0:c1],
                    start=False,
                    stop=True,
                ).then_inc(s_p, 1)

        @block.scalar
        def _(a):
            for i, ci in enumerate(chunk_order):
                c0 = ci * CHUNK
                c1 = min(c0 + CHUNK, NODES_PAD)
                wdt = c1 - c0
                a.wait_ge(s_p, i + 1)
                if b_l == 0.0:
                    ins = a.activation(
                        out=orow[:, c0:c1],
                        in_=ps[i % NPSUM][:, :wdt],
                        func=mybir.ActivationFunctionType.Copy,
                    )
                else:
                    ins = a.activation(
                        out=orow[:, c0:c1],
                        in_=ps[i % NPSUM][:, :wdt],
                        func=mybir.ActivationFunctionType.Identity,
                        bias=float(b_l),
                    )
                ins.then_inc(s_ad, 1)

    return nc


# ---------------------------------------------------------------- entry
def _run(inputs, trace=False, trace_cores=None):
    from concourse.bass_utils import run_bass_kernel_spmd

    in_maps, orders, K_prog, b_l = _preprocess(**inputs)
    nc = _build_program(K_prog, b_l)
    res = run_bass_kernel_spmd(
        nc,
        in_maps,
        core_ids=list(range(N_CORES)),
        trace=trace,
        trace_cores=trace_cores,
    )
    return _assemble(res.results, orders), res


def kernel(**inputs):
    out, _ = _run(inputs)
    return out
